# revision 16
# baseline (speedup 1.0000x reference)
"""Self-contained Trainium2 Bass kernel: multi-head attention (B=4, N=2048, C=1024, H=16).

Sharding: 8 cores = 4 batches x 2 query-halves (pure data/sequence parallel,
no collectives). Each core computes q for its 1024 query rows, full k/v for
its batch (KV projection duplicated across the 2 cores of a batch), the
attention for all 16 heads on its query half, and the output projection for
its rows. Host assembles the 8 disjoint [1024, 1024] row slices.

The host rotates each core's x^T so its own query half is always columns
0:1024 (attention is permutation invariant over keys as long as k and v use
the same key order), which keeps the program SPMD-identical across cores.

All matmuls bf16 with fp32 PSUM accumulation. Softmax skips the max
subtraction (scores are ~N(0,1) after the 1/8 scale folded into Wq; verified
|S| < 7 so exp cannot overflow). Row sums ride the AV matmul via a ones
column appended to V. S^T is computed with two heads row-packed in the PE
array (K=64 each, tile_position (0,0)/(64,0)). The q/k projections of pair
j+1 are woven between the attention units of pair j so the PE stream stays
dense while ACT (exp) is the per-unit bottleneck. Normalization is batched
at the end (one reciprocal, DMA partition-broadcast via DRAM); the
unnormalized O^T aliases the dead q^T tiles and the normalized A^T aliases
the dead k^T tiles to fit SBUF.
"""

import numpy as np
import ml_dtypes

B, N, C, H = 4, 2048, 1024, 16
DH = C // H                      # 64
SCALE = DH ** -0.5
NCORES = 8
QH = N // 2                      # 1024 query rows per core
MT = N // 128                    # 16 key tiles
CT = C // 128                    # 8 contraction tiles
DT = C // 128                    # 8 d tiles
NQC = QH // 512                  # 2 query chunks of 512

_BF16 = ml_dtypes.bfloat16
_cache = {}


def _patch_tile_drain():
    """Walrus in this env rejects >1 sem wait per instruction; split the tail
    Drain's waits into standalone single-wait nops on SP."""
    import concourse.tile as tile
    import concourse.mybir as mybir
    from concourse.vector_clock import ScopedClock

    if getattr(tile.TileContext, "_drain_split_patched", False):
        return

    def _patched(self, tick_clock, wait_clock):
        nc = self.nc
        drain_inst = nc.sync.drain()
        wait_clock.add_sem_waits(
            drain_inst.ins, ScopedClock({None: tick_clock.global_clock})
        )
        si = drain_inst.ins.sync_info
        waits = list(si.on_wait) if si is not None and si.on_wait else []
        if len(waits) > 1:
            si.on_wait = []
            for w in waits:
                nop = nc.sync.nop(hint="drain_wait_split", nofuse=True)
                nsi = nop.ins.sync_info
                if nsi is None:
                    nop.ins.sync_info = mybir.SyncInfo(on_wait=[w], on_update=[])
                else:
                    nsi.on_wait = [w]
        nc.all_engine_barrier()
        assert self.sems is not None
        popped = nc._tile_sem_poison_stack.pop()
        assert popped is self._sem_poison
        nc.clear_and_free_semaphores(list(self.sems.allocated().values()))
        nc.all_engine_barrier()

    tile.TileContext._drain_and_barrier = _patched
    tile.TileContext._drain_split_patched = True


def _split_excess_waits(nc, limit=1):
    """Walrus here rejects instructions carrying more than `limit` sem waits.
    Move the excess onto same-engine nops inserted immediately before."""
    import concourse.mybir as mybir

    counter = [0]
    for block in nc.m.functions[0].blocks:
        il = block.instructions
        i = 0
        while i < len(il):
            inst = il[i]
            si = inst.sync_info
            waits = list(si.on_wait) if si is not None and si.on_wait else []
            if len(waits) > limit:
                keep = waits[-limit:]
                extra = waits[:-limit]
                si.on_wait = keep
                pos = i
                for j in range(0, len(extra), limit):
                    chunk = extra[j : j + limit]
                    counter[0] += 1
                    nop = mybir.InstNoOp(
                        name=f"waitsplit_{counter[0]}",
                        engine=inst.engine,
                        ins=[],
                        outs=[],
                        sync_info=mybir.SyncInfo(on_wait=chunk, on_update=[]),
                    )
                    try:
                        nc.register_instruction(nop, overwrite=True)
                    except Exception:
                        pass
                    il.insert(pos, nop)
                    pos += 1
                    i += 1
            i += 1


def build_nc():
    import concourse.bass as bass
    import concourse.mybir as mybir
    import concourse.tile as tile

    _patch_tile_drain()
    f32 = mybir.dt.float32
    bf16 = mybir.dt.bfloat16
    EXP = mybir.ActivationFunctionType.Exp

    nc = bass.Bass("TRN2", num_devices=NCORES)
    xT = nc.dram_tensor("xT", [C, N], bf16, kind="ExternalInput")
    Wq = nc.dram_tensor("Wq", [C, C], bf16, kind="ExternalInput")
    Wk = nc.dram_tensor("Wk", [C, C], bf16, kind="ExternalInput")
    Wv = nc.dram_tensor("Wv", [C, C], bf16, kind="ExternalInput")
    Wout = nc.dram_tensor("Wout", [C, C], bf16, kind="ExternalInput")
    bout = nc.dram_tensor("bout", [128, C], f32, kind="ExternalInput")
    y = nc.dram_tensor("y", [QH, C], f32, kind="ExternalOutput")

    with tile.TileContext(nc) as tc:
      with tc.tile_pool(name="persist", bufs=1) as persist:
        qT_sb = [persist.tile([128, QH], bf16, name=f"qT{j}", tag=f"qT{j}") for j in range(DT)]
        kT_sb = [persist.tile([128, N], bf16, name=f"kT{j}", tag=f"kT{j}") for j in range(DT)]
        v_sb = [persist.tile([128, H, DH + 1], bf16, name=f"v{m}", tag=f"v{m}") for m in range(MT)]
        rs_all = persist.tile([H, QH], f32)
        # aliases: after S of pair j consumes qT[j]/kT[j], their SBUF holds
        # the unnormalized O^T and the normalized A^T
        OT_un = qT_sb
        OT_fin = [kT_sb[j][:, 0:QH] for j in range(DT)]

        with (
            tc.tile_pool(name="projA", bufs=1) as projA,
            tc.tile_pool(name="ps_pp", bufs=2, space="PSUM") as pp,
        ):
            xT_t = projA.tile([128, CT, N], bf16)
            xT_r = xT.ap().rearrange("(a p) n -> p a n", p=128)
            for blk in range(MT):
                bs = slice(blk * 128, (blk + 1) * 128)
                nc.sync.dma_start(out=xT_t[:, :, bs], in_=xT_r[:, :, bs])
            Wq_t = projA.tile([128, CT, C], bf16)
            nc.sync.dma_start(
                out=Wq_t, in_=Wq.ap().rearrange("(a p) d -> p a d", p=128)
            )
            Wk_t = projA.tile([128, CT, C], bf16)
            nc.sync.dma_start(
                out=Wk_t, in_=Wk.ap().rearrange("(a p) d -> p a d", p=128)
            )

            def emit_v(Wv_t, mt):
                nc.vector.memset(v_sb[mt][:, :, DH : DH + 1], 1.0)
                for ch in range(2):
                    psv = pp.tile([128, 512], f32, tag="ps", name=f"psv{mt}_{ch}")
                    for jc in range(CT):
                        nc.tensor.matmul(
                            psv,
                            xT_t[:, jc, mt * 128 : (mt + 1) * 128],
                            Wv_t[:, jc, ch * 512 : (ch + 1) * 512],
                            start=(jc == 0),
                            stop=(jc == CT - 1),
                        )
                    nc.vector.tensor_copy(
                        out=v_sb[mt][:, ch * 8 : (ch + 1) * 8, 0:DH],
                        in_=psv.rearrange("p (h d) -> p h d", h=8),
                    )

            def emit_q(jd, ch):
                psq = pp.tile([128, 512], f32, tag="ps", name=f"psq{jd}_{ch}")
                for jc in range(CT):
                    nc.tensor.matmul(
                        psq,
                        Wq_t[:, jc, jd * 128 : (jd + 1) * 128],
                        xT_t[:, jc, ch * 512 : (ch + 1) * 512],
                        start=(jc == 0),
                        stop=(jc == CT - 1),
                    )
                nc.vector.tensor_copy(
                    out=qT_sb[jd][:, ch * 512 : (ch + 1) * 512], in_=psq
                )

            def emit_k(jd, ch):
                psk = pp.tile([128, 512], f32, tag="ps", name=f"psk{jd}_{ch}")
                for jc in range(CT):
                    nc.tensor.matmul(
                        psk,
                        Wk_t[:, jc, jd * 128 : (jd + 1) * 128],
                        xT_t[:, jc, ch * 512 : (ch + 1) * 512],
                        start=(jc == 0),
                        stop=(jc == CT - 1),
                    )
                nc.vector.tensor_copy(
                    out=kT_sb[jd][:, ch * 512 : (ch + 1) * 512], in_=psk
                )

            def qk_ops(jd, half):
                # half 0: both q chunks + first k chunk; half 1: rest of k
                if half == 0:
                    return [lambda: emit_q(jd, 0), lambda: emit_q(jd, 1),
                            lambda: emit_k(jd, 0)]
                return [lambda ch=ch: emit_k(jd, ch) for ch in range(1, 4)]

            def emit_qk(jd, half):
                for op in qk_ops(jd, half):
                    op()

            with tc.tile_pool(name="projV", bufs=1) as projV:
                Wv_t = projV.tile([128, CT, C], bf16)
                nc.sync.dma_start(
                    out=Wv_t, in_=Wv.ap().rearrange("(a p) d -> p a d", p=128)
                )
                for mt in range(MT):
                    emit_v(Wv_t, mt)
            emit_qk(0, 0)
            emit_qk(0, 1)

            # ------------- attention (with woven q/k projections) ----------
            with (
                tc.tile_pool(name="aw_big", bufs=1) as awb,
                tc.tile_pool(name="aw_small", bufs=3) as aws,
                tc.tile_pool(name="ps_st", bufs=2, space="PSUM") as ps_st,
                tc.tile_pool(name="ps_ot", bufs=2, space="PSUM") as ps_ot,
            ):
                for pr in range(H // 2):      # head pairs; d-tile jd == pr
                    he, ho = 2 * pr, 2 * pr + 1
                    for qc in range(NQC):
                        qs = slice(qc * 512, (qc + 1) * 512)
                        weave = qk_ops(pr + 1, qc) if pr + 1 < DT else []
                        pt = awb.tile([128, MT, 1024], bf16, tag="pt")
                        ot_e = ps_ot.tile([65, 512], f32, tag="ot")
                        ot_o = ps_ot.tile([65, 512], f32, tag="ot")
                        for mt in range(MT):
                            if weave and mt % 5 == 4:
                                weave.pop(0)()
                            ms = slice(mt * 128, (mt + 1) * 128)
                            st = ps_st.tile([128, 1024], f32, tag="st")
                            # S^T tiles for both heads, row-packed (K=64)
                            nc.tensor.matmul(
                                st[:, 0:512],
                                kT_sb[pr][0:64, ms],
                                qT_sb[pr][0:64, qs],
                                start=True, stop=True,
                                tile_position=(0, 0),
                            )
                            nc.tensor.matmul(
                                st[:, 512:1024],
                                kT_sb[pr][64:128, ms],
                                qT_sb[pr][64:128, qs],
                                start=True, stop=True,
                                tile_position=(64, 0),
                            )
                            nc.scalar.activation(out=pt[:, mt, :], in_=st, func=EXP)
                            nc.tensor.matmul(
                                ot_e,
                                v_sb[mt][:, he, :],
                                pt[:, mt, 0:512],
                                start=(mt == 0), stop=(mt == MT - 1),
                            )
                            nc.tensor.matmul(
                                ot_o,
                                v_sb[mt][:, ho, :],
                                pt[:, mt, 512:1024],
                                start=(mt == 0), stop=(mt == MT - 1),
                            )
                        # stash unnormalized O^T (bf16, aliases qT) + rowsums
                        for po, h, ot in ((0, he, ot_e), (64, ho, ot_o)):
                            tmp = aws.tile([64, 512], bf16, tag="tmp")
                            nc.vector.tensor_copy(out=tmp, in_=ot[0:64, :])
                            nc.sync.dma_start(
                                out=OT_un[pr][po : po + 64, qs], in_=tmp
                            )
                            rsv = aws.tile([65, 512], f32, tag="rsv")
                            nc.vector.tensor_copy(
                                out=rsv[64:65, :], in_=ot[64:65, :]
                            )
                            nc.sync.dma_start(
                                out=rs_all[h : h + 1, qs], in_=rsv[64:65, :]
                            )
                        for op in weave:
                            op()

        # ------------- batched normalization + output projection ----------
        with (
            tc.tile_pool(name="normp", bufs=2) as np_pool,
            tc.tile_pool(name="dramp", bufs=1, space="DRAM") as dram_pool,
            tc.tile_pool(name="yout", bufs=2) as yp,
            tc.tile_pool(name="ps_y", bufs=2, space="PSUM") as ps_y,
        ):
            Wout_t = yp.tile([128, DT, C], bf16, bufs=1)
            nc.sync.dma_start(
                out=Wout_t, in_=Wout.ap().rearrange("(a p) d -> p a d", p=128)
            )
            bout_t = yp.tile([128, C], f32, bufs=1)
            nc.sync.dma_start(out=bout_t, in_=bout.ap())

            rinv_all = np_pool.tile([H, QH], f32, bufs=1)
            nc.vector.reciprocal(out=rinv_all, in_=rs_all)
            rinv_dram = dram_pool.tile([H, QH], f32)
            nc.sync.dma_start(out=rinv_dram, in_=rinv_all)
            for jd in range(DT):
                rbc = np_pool.tile([128, QH], f32, tag="rbc")
                nc.sync.dma_start(
                    out=rbc[0:64, :],
                    in_=rinv_dram[2 * jd : 2 * jd + 1, :].to_broadcast([64, QH]),
                )
                nc.sync.dma_start(
                    out=rbc[64:128, :],
                    in_=rinv_dram[2 * jd + 1 : 2 * jd + 2, :].to_broadcast([64, QH]),
                )
                nc.vector.tensor_mul(OT_fin[jd], OT_un[jd], rbc)

            for t in range(QH // 128):
                psy = ps_y.tile([128, C], f32, tag="y")
                for jd in range(DT):
                    for ch in range(2):
                        nc.tensor.matmul(
                            psy[:, ch * 512 : (ch + 1) * 512],
                            OT_fin[jd][:, t * 128 : (t + 1) * 128],
                            Wout_t[:, jd, ch * 512 : (ch + 1) * 512],
                            start=(jd == 0),
                            stop=(jd == DT - 1),
                        )
                ys = yp.tile([128, C], f32, tag="ys")
                nc.vector.tensor_add(ys, psy, bout_t)
                nc.sync.dma_start(out=y[t * 128 : (t + 1) * 128, :], in_=ys)
    _split_excess_waits(nc)
    return nc


def make_in_maps(x, Wq, Wkv, Wout, bout):
    x = np.asarray(x, dtype=np.float32)
    Wq = np.asarray(Wq, dtype=np.float32)
    Wkv = np.asarray(Wkv, dtype=np.float32)
    Wout = np.asarray(Wout, dtype=np.float32)
    bout = np.asarray(bout, dtype=np.float32)
    Wq_b = np.ascontiguousarray((Wq * SCALE)).astype(_BF16)
    Wk_b = np.ascontiguousarray(Wkv[:, :C]).astype(_BF16)
    Wv_b = np.ascontiguousarray(Wkv[:, C:]).astype(_BF16)
    Wout_b = np.ascontiguousarray(Wout).astype(_BF16)
    bout_bc = np.ascontiguousarray(np.broadcast_to(bout, (128, C))).astype(np.float32)
    in_maps = []
    for core in range(NCORES):
        b, g = core // 2, core % 2
        # rotate so this core's query half is always columns 0:QH of xT
        xrot = np.concatenate(
            [x[b, g * QH : (g + 1) * QH], x[b, (1 - g) * QH : (2 - g) * QH]],
            axis=0,
        )
        xT_r = np.ascontiguousarray(xrot.T).astype(_BF16)
        in_maps.append(
            dict(xT=xT_r, Wq=Wq_b, Wk=Wk_b, Wv=Wv_b, Wout=Wout_b, bout=bout_bc)
        )
    return in_maps


def assemble(results):
    out = np.empty((B, N, C), dtype=np.float32)
    for core in range(NCORES):
        b, g = core // 2, core % 2
        out[b, g * QH : (g + 1) * QH, :] = results[core]["y"]
    return out


def kernel(x, Wq, Wkv, Wout, bout):
    from concourse.bass_utils import run_bass_kernel_spmd

    if "nc" not in _cache:
        _cache["nc"] = build_nc()
    in_maps = make_in_maps(x, Wq, Wkv, Wout, bout)
    res = run_bass_kernel_spmd(_cache["nc"], in_maps, core_ids=list(range(NCORES)))
    return assemble(res.results)


# revision 17
# speedup vs baseline: 1.0389x; 1.0389x over previous
"""Self-contained Trainium2 Bass kernel: multi-head attention (B=4, N=2048, C=1024, H=16).

Sharding: 8 cores = 4 batches x 2 query-halves (pure data/sequence parallel,
no collectives). Each core computes q for its 1024 query rows, full k/v for
its batch (KV projection duplicated across the 2 cores of a batch), the
attention for all 16 heads on its query half, and the output projection for
its rows. Host assembles the 8 disjoint [1024, 1024] row slices.

The host rotates each core's x^T so its own query half is always columns
0:1024 (attention is permutation invariant over keys as long as k and v use
the same key order), which keeps the program SPMD-identical across cores.

All matmuls bf16 with fp32 PSUM accumulation. Softmax skips the max
subtraction (scores are ~N(0,1) after the 1/8 scale folded into Wq; verified
|S| < 7 so exp cannot overflow). Row sums ride the AV matmul via a ones
column appended to V. S^T is computed with two heads row-packed in the PE
array (K=64 each, tile_position (0,0)/(64,0)). The q/k projections of pair
j+1 are woven between the attention units of pair j so the PE stream stays
dense while ACT (exp) is the per-unit bottleneck. Normalization is batched
at the end (one reciprocal, DMA partition-broadcast via DRAM); the
unnormalized O^T aliases the dead q^T tiles and the normalized A^T aliases
the dead k^T tiles to fit SBUF.
"""

import numpy as np
import ml_dtypes

B, N, C, H = 4, 2048, 1024, 16
DH = C // H                      # 64
SCALE = DH ** -0.5
NCORES = 8
QH = N // 2                      # 1024 query rows per core
MT = N // 128                    # 16 key tiles
CT = C // 128                    # 8 contraction tiles
DT = C // 128                    # 8 d tiles
NQC = QH // 512                  # 2 query chunks of 512

_BF16 = ml_dtypes.bfloat16
_cache = {}


def _patch_tile_drain():
    """Walrus in this env rejects >1 sem wait per instruction; split the tail
    Drain's waits into standalone single-wait nops on SP."""
    import concourse.tile as tile
    import concourse.mybir as mybir
    from concourse.vector_clock import ScopedClock

    if getattr(tile.TileContext, "_drain_split_patched", False):
        return

    def _patched(self, tick_clock, wait_clock):
        nc = self.nc
        drain_inst = nc.sync.drain()
        wait_clock.add_sem_waits(
            drain_inst.ins, ScopedClock({None: tick_clock.global_clock})
        )
        si = drain_inst.ins.sync_info
        waits = list(si.on_wait) if si is not None and si.on_wait else []
        if len(waits) > 1:
            si.on_wait = []
            for w in waits:
                nop = nc.sync.nop(hint="drain_wait_split", nofuse=True)
                nsi = nop.ins.sync_info
                if nsi is None:
                    nop.ins.sync_info = mybir.SyncInfo(on_wait=[w], on_update=[])
                else:
                    nsi.on_wait = [w]
        nc.all_engine_barrier()
        assert self.sems is not None
        popped = nc._tile_sem_poison_stack.pop()
        assert popped is self._sem_poison
        nc.clear_and_free_semaphores(list(self.sems.allocated().values()))
        nc.all_engine_barrier()

    tile.TileContext._drain_and_barrier = _patched
    tile.TileContext._drain_split_patched = True


def _split_excess_waits(nc, limit=1):
    """Walrus here rejects instructions carrying more than `limit` sem waits.
    Move the excess onto same-engine nops inserted immediately before."""
    import concourse.mybir as mybir

    counter = [0]
    for block in nc.m.functions[0].blocks:
        il = block.instructions
        i = 0
        while i < len(il):
            inst = il[i]
            si = inst.sync_info
            waits = list(si.on_wait) if si is not None and si.on_wait else []
            if len(waits) > limit:
                keep = waits[-limit:]
                extra = waits[:-limit]
                si.on_wait = keep
                pos = i
                for j in range(0, len(extra), limit):
                    chunk = extra[j : j + limit]
                    counter[0] += 1
                    nop = mybir.InstNoOp(
                        name=f"waitsplit_{counter[0]}",
                        engine=inst.engine,
                        ins=[],
                        outs=[],
                        sync_info=mybir.SyncInfo(on_wait=chunk, on_update=[]),
                    )
                    try:
                        nc.register_instruction(nop, overwrite=True)
                    except Exception:
                        pass
                    il.insert(pos, nop)
                    pos += 1
                    i += 1
            i += 1


def build_nc():
    import concourse.bass as bass
    import concourse.mybir as mybir
    import concourse.tile as tile

    _patch_tile_drain()
    f32 = mybir.dt.float32
    bf16 = mybir.dt.bfloat16
    EXP = mybir.ActivationFunctionType.Exp

    nc = bass.Bass("TRN2", num_devices=NCORES)
    xT = nc.dram_tensor("xT", [C, N], bf16, kind="ExternalInput")
    Wq = nc.dram_tensor("Wq", [C, C], bf16, kind="ExternalInput")
    Wk = nc.dram_tensor("Wk", [C, C], bf16, kind="ExternalInput")
    Wv = nc.dram_tensor("Wv", [C, C], bf16, kind="ExternalInput")
    Wout = nc.dram_tensor("Wout", [C, C], bf16, kind="ExternalInput")
    bout = nc.dram_tensor("bout", [128, C], f32, kind="ExternalInput")
    y = nc.dram_tensor("y", [QH, C], f32, kind="ExternalOutput")

    with tile.TileContext(nc) as tc:
      with tc.tile_pool(name="persist", bufs=1) as persist:
        qT_sb = [persist.tile([128, QH], bf16, name=f"qT{j}", tag=f"qT{j}") for j in range(DT)]
        kT_sb = [persist.tile([128, N], bf16, name=f"kT{j}", tag=f"kT{j}") for j in range(DT)]
        v_sb = [persist.tile([128, H, DH + 1], bf16, name=f"v{m}", tag=f"v{m}") for m in range(MT)]
        rs_all = persist.tile([H, QH], f32)
        # aliases: after S of pair j consumes qT[j]/kT[j], their SBUF holds
        # the unnormalized O^T and the normalized A^T
        OT_un = qT_sb
        OT_fin = [kT_sb[j][:, 0:QH] for j in range(DT)]

        with (
            tc.tile_pool(name="projA", bufs=1) as projA,
            tc.tile_pool(name="ps_pp", bufs=2, space="PSUM") as pp,
        ):
            xT_t = projA.tile([128, CT, N], bf16)
            nc.sync.dma_start(
                out=xT_t, in_=xT.ap().rearrange("(a p) n -> p a n", p=128)
            )
            Wq_t = projA.tile([128, CT, C], bf16)
            nc.sync.dma_start(
                out=Wq_t, in_=Wq.ap().rearrange("(a p) d -> p a d", p=128)
            )
            Wk_t = projA.tile([128, CT, C], bf16)
            nc.sync.dma_start(
                out=Wk_t, in_=Wk.ap().rearrange("(a p) d -> p a d", p=128)
            )

            def emit_v(Wv_t, mt):
                nc.vector.memset(v_sb[mt][:, :, DH : DH + 1], 1.0)
                for ch in range(2):
                    psv = pp.tile([128, 512], f32, tag="ps", name=f"psv{mt}_{ch}")
                    for jc in range(CT):
                        nc.tensor.matmul(
                            psv,
                            xT_t[:, jc, mt * 128 : (mt + 1) * 128],
                            Wv_t[:, jc, ch * 512 : (ch + 1) * 512],
                            start=(jc == 0),
                            stop=(jc == CT - 1),
                        )
                    nc.vector.tensor_copy(
                        out=v_sb[mt][:, ch * 8 : (ch + 1) * 8, 0:DH],
                        in_=psv.rearrange("p (h d) -> p h d", h=8),
                    )

            def emit_q(jd, ch):
                psq = pp.tile([128, 512], f32, tag="ps", name=f"psq{jd}_{ch}")
                for jc in range(CT):
                    nc.tensor.matmul(
                        psq,
                        Wq_t[:, jc, jd * 128 : (jd + 1) * 128],
                        xT_t[:, jc, ch * 512 : (ch + 1) * 512],
                        start=(jc == 0),
                        stop=(jc == CT - 1),
                    )
                nc.vector.tensor_copy(
                    out=qT_sb[jd][:, ch * 512 : (ch + 1) * 512], in_=psq
                )

            def emit_k(jd, ch):
                psk = pp.tile([128, 512], f32, tag="ps", name=f"psk{jd}_{ch}")
                for jc in range(CT):
                    nc.tensor.matmul(
                        psk,
                        Wk_t[:, jc, jd * 128 : (jd + 1) * 128],
                        xT_t[:, jc, ch * 512 : (ch + 1) * 512],
                        start=(jc == 0),
                        stop=(jc == CT - 1),
                    )
                nc.vector.tensor_copy(
                    out=kT_sb[jd][:, ch * 512 : (ch + 1) * 512], in_=psk
                )

            def qk_ops(jd, half):
                # half 0: both q chunks + first k chunk; half 1: rest of k
                if half == 0:
                    return [lambda: emit_q(jd, 0), lambda: emit_q(jd, 1),
                            lambda: emit_k(jd, 0)]
                return [lambda ch=ch: emit_k(jd, ch) for ch in range(1, 4)]

            def emit_qk(jd, half):
                for op in qk_ops(jd, half):
                    op()

            with tc.tile_pool(name="projV", bufs=1) as projV:
                Wv_t = projV.tile([128, CT, C], bf16)
                nc.sync.dma_start(
                    out=Wv_t, in_=Wv.ap().rearrange("(a p) d -> p a d", p=128)
                )
                for mt in range(MT):
                    emit_v(Wv_t, mt)
            emit_qk(0, 0)
            emit_qk(0, 1)

            # ------------- attention (with woven q/k projections) ----------
            with (
                tc.tile_pool(name="aw_big", bufs=1) as awb,
                tc.tile_pool(name="aw_small", bufs=3) as aws,
                tc.tile_pool(name="ps_st", bufs=2, space="PSUM") as ps_st,
                tc.tile_pool(name="ps_ot", bufs=2, space="PSUM") as ps_ot,
            ):
                for pr in range(H // 2):      # head pairs; d-tile jd == pr
                    he, ho = 2 * pr, 2 * pr + 1
                    for qc in range(NQC):
                        qs = slice(qc * 512, (qc + 1) * 512)
                        pt = awb.tile([128, MT, 1024], bf16, tag="pt")
                        ot_e = ps_ot.tile([65, 512], f32, tag="ot")
                        ot_o = ps_ot.tile([65, 512], f32, tag="ot")
                        for mt in range(MT):
                            ms = slice(mt * 128, (mt + 1) * 128)
                            st = ps_st.tile([128, 1024], f32, tag="st")
                            # S^T tiles for both heads, row-packed (K=64)
                            nc.tensor.matmul(
                                st[:, 0:512],
                                kT_sb[pr][0:64, ms],
                                qT_sb[pr][0:64, qs],
                                start=True, stop=True,
                                tile_position=(0, 0),
                            )
                            nc.tensor.matmul(
                                st[:, 512:1024],
                                kT_sb[pr][64:128, ms],
                                qT_sb[pr][64:128, qs],
                                start=True, stop=True,
                                tile_position=(64, 0),
                            )
                            nc.scalar.activation(out=pt[:, mt, :], in_=st, func=EXP)
                            nc.tensor.matmul(
                                ot_e,
                                v_sb[mt][:, he, :],
                                pt[:, mt, 0:512],
                                start=(mt == 0), stop=(mt == MT - 1),
                            )
                            nc.tensor.matmul(
                                ot_o,
                                v_sb[mt][:, ho, :],
                                pt[:, mt, 512:1024],
                                start=(mt == 0), stop=(mt == MT - 1),
                            )
                        # stash unnormalized O^T (bf16, aliases qT) + rowsums
                        for po, h, ot in ((0, he, ot_e), (64, ho, ot_o)):
                            tmp = aws.tile([64, 512], bf16, tag="tmp")
                            nc.vector.tensor_copy(out=tmp, in_=ot[0:64, :])
                            nc.sync.dma_start(
                                out=OT_un[pr][po : po + 64, qs], in_=tmp
                            )
                            rsv = aws.tile([65, 512], f32, tag="rsv")
                            nc.vector.tensor_copy(
                                out=rsv[64:65, :], in_=ot[64:65, :]
                            )
                            nc.sync.dma_start(
                                out=rs_all[h : h + 1, qs], in_=rsv[64:65, :]
                            )
                        # weave the next pair's q/k projection
                        if pr + 1 < DT:
                            emit_qk(pr + 1, qc)

        # ------------- batched normalization + output projection ----------
        with (
            tc.tile_pool(name="normp", bufs=2) as np_pool,
            tc.tile_pool(name="dramp", bufs=1, space="DRAM") as dram_pool,
            tc.tile_pool(name="yout", bufs=2) as yp,
            tc.tile_pool(name="ps_y", bufs=2, space="PSUM") as ps_y,
        ):
            Wout_t = yp.tile([128, DT, C], bf16, bufs=1)
            nc.sync.dma_start(
                out=Wout_t, in_=Wout.ap().rearrange("(a p) d -> p a d", p=128)
            )
            bout_t = yp.tile([128, C], f32, bufs=1)
            nc.sync.dma_start(out=bout_t, in_=bout.ap())

            rinv_all = np_pool.tile([H, QH], f32, bufs=1)
            nc.vector.reciprocal(out=rinv_all, in_=rs_all)
            rinv_dram = dram_pool.tile([H, QH], f32)
            nc.sync.dma_start(out=rinv_dram, in_=rinv_all)
            for jd in range(DT):
                rbc = np_pool.tile([128, QH], f32, tag="rbc")
                nc.sync.dma_start(
                    out=rbc[0:64, :],
                    in_=rinv_dram[2 * jd : 2 * jd + 1, :].to_broadcast([64, QH]),
                )
                nc.sync.dma_start(
                    out=rbc[64:128, :],
                    in_=rinv_dram[2 * jd + 1 : 2 * jd + 2, :].to_broadcast([64, QH]),
                )
                nc.vector.tensor_mul(OT_fin[jd], OT_un[jd], rbc)

            for t in range(QH // 128):
                psy = ps_y.tile([128, C], f32, tag="y")
                for jd in range(DT):
                    for ch in range(2):
                        nc.tensor.matmul(
                            psy[:, ch * 512 : (ch + 1) * 512],
                            OT_fin[jd][:, t * 128 : (t + 1) * 128],
                            Wout_t[:, jd, ch * 512 : (ch + 1) * 512],
                            start=(jd == 0),
                            stop=(jd == DT - 1),
                        )
                ys = yp.tile([128, C], f32, tag="ys")
                nc.vector.tensor_add(ys, psy, bout_t)
                nc.sync.dma_start(out=y[t * 128 : (t + 1) * 128, :], in_=ys)
    _split_excess_waits(nc)
    return nc


def make_in_maps(x, Wq, Wkv, Wout, bout):
    x = np.asarray(x, dtype=np.float32)
    Wq = np.asarray(Wq, dtype=np.float32)
    Wkv = np.asarray(Wkv, dtype=np.float32)
    Wout = np.asarray(Wout, dtype=np.float32)
    bout = np.asarray(bout, dtype=np.float32)
    Wq_b = np.ascontiguousarray((Wq * SCALE)).astype(_BF16)
    Wk_b = np.ascontiguousarray(Wkv[:, :C]).astype(_BF16)
    Wv_b = np.ascontiguousarray(Wkv[:, C:]).astype(_BF16)
    Wout_b = np.ascontiguousarray(Wout).astype(_BF16)
    bout_bc = np.ascontiguousarray(np.broadcast_to(bout, (128, C))).astype(np.float32)
    in_maps = []
    for core in range(NCORES):
        b, g = core // 2, core % 2
        # rotate so this core's query half is always columns 0:QH of xT
        xrot = np.concatenate(
            [x[b, g * QH : (g + 1) * QH], x[b, (1 - g) * QH : (2 - g) * QH]],
            axis=0,
        )
        xT_r = np.ascontiguousarray(xrot.T).astype(_BF16)
        in_maps.append(
            dict(xT=xT_r, Wq=Wq_b, Wk=Wk_b, Wv=Wv_b, Wout=Wout_b, bout=bout_bc)
        )
    return in_maps


def assemble(results):
    out = np.empty((B, N, C), dtype=np.float32)
    for core in range(NCORES):
        b, g = core // 2, core % 2
        out[b, g * QH : (g + 1) * QH, :] = results[core]["y"]
    return out


def kernel(x, Wq, Wkv, Wout, bout):
    from concourse.bass_utils import run_bass_kernel_spmd

    if "nc" not in _cache:
        _cache["nc"] = build_nc()
    in_maps = make_in_maps(x, Wq, Wkv, Wout, bout)
    res = run_bass_kernel_spmd(_cache["nc"], in_maps, core_ids=list(range(NCORES)))
    return assemble(res.results)


# revision 19
# speedup vs baseline: 1.0810x; 1.0405x over previous
"""Self-contained Trainium2 Bass kernel: multi-head attention (B=4, N=2048, C=1024, H=16).

Sharding: 8 cores = 4 batches x 2 query-halves (pure data/sequence parallel,
no collectives). Each core computes q for its 1024 query rows, full k/v for
its batch (KV projection duplicated across the 2 cores of a batch), the
attention for all 16 heads on its query half, and the output projection for
its rows. Host assembles the 8 disjoint [1024, 1024] row slices.

The host rotates each core's x^T so its own query half is always columns
0:1024 (attention is permutation invariant over keys as long as k and v use
the same key order), which keeps the program SPMD-identical across cores.

All matmuls bf16 with fp32 PSUM accumulation. Softmax skips the max
subtraction (scores are ~N(0,1) after the 1/8 scale folded into Wq; verified
|S| < 7 so exp cannot overflow). Row sums ride the AV matmul via a ones
column appended to V. S^T is computed with two heads row-packed in the PE
array (K=64 each, tile_position (0,0)/(64,0)). The q/k projections of pair
j+1 are woven between the attention units of pair j so the PE stream stays
dense while ACT (exp) is the per-unit bottleneck. Normalization is batched
at the end (one reciprocal, DMA partition-broadcast via DRAM); the
unnormalized O^T aliases the dead q^T tiles and the normalized A^T aliases
the dead k^T tiles to fit SBUF.
"""

import numpy as np
import ml_dtypes

B, N, C, H = 4, 2048, 1024, 16
DH = C // H                      # 64
SCALE = DH ** -0.5
NCORES = 8
QH = N // 2                      # 1024 query rows per core
MT = N // 128                    # 16 key tiles
CT = C // 128                    # 8 contraction tiles
DT = C // 128                    # 8 d tiles
NQC = QH // 512                  # 2 query chunks of 512

_BF16 = ml_dtypes.bfloat16
_cache = {}


def _patch_tile_drain():
    """Walrus in this env rejects >1 sem wait per instruction; split the tail
    Drain's waits into standalone single-wait nops on SP."""
    import concourse.tile as tile
    import concourse.mybir as mybir
    from concourse.vector_clock import ScopedClock

    if getattr(tile.TileContext, "_drain_split_patched", False):
        return

    def _patched(self, tick_clock, wait_clock):
        nc = self.nc
        drain_inst = nc.sync.drain()
        wait_clock.add_sem_waits(
            drain_inst.ins, ScopedClock({None: tick_clock.global_clock})
        )
        si = drain_inst.ins.sync_info
        waits = list(si.on_wait) if si is not None and si.on_wait else []
        if len(waits) > 1:
            si.on_wait = []
            for w in waits:
                nop = nc.sync.nop(hint="drain_wait_split", nofuse=True)
                nsi = nop.ins.sync_info
                if nsi is None:
                    nop.ins.sync_info = mybir.SyncInfo(on_wait=[w], on_update=[])
                else:
                    nsi.on_wait = [w]
        nc.all_engine_barrier()
        assert self.sems is not None
        popped = nc._tile_sem_poison_stack.pop()
        assert popped is self._sem_poison
        nc.clear_and_free_semaphores(list(self.sems.allocated().values()))
        nc.all_engine_barrier()

    tile.TileContext._drain_and_barrier = _patched
    tile.TileContext._drain_split_patched = True


def _split_excess_waits(nc, limit=1):
    """Walrus here rejects instructions carrying more than `limit` sem waits.
    Move the excess onto same-engine nops inserted immediately before."""
    import concourse.mybir as mybir

    counter = [0]
    for block in nc.m.functions[0].blocks:
        il = block.instructions
        i = 0
        while i < len(il):
            inst = il[i]
            si = inst.sync_info
            waits = list(si.on_wait) if si is not None and si.on_wait else []
            if len(waits) > limit:
                keep = waits[-limit:]
                extra = waits[:-limit]
                si.on_wait = keep
                pos = i
                for j in range(0, len(extra), limit):
                    chunk = extra[j : j + limit]
                    counter[0] += 1
                    nop = mybir.InstNoOp(
                        name=f"waitsplit_{counter[0]}",
                        engine=inst.engine,
                        ins=[],
                        outs=[],
                        sync_info=mybir.SyncInfo(on_wait=chunk, on_update=[]),
                    )
                    try:
                        nc.register_instruction(nop, overwrite=True)
                    except Exception:
                        pass
                    il.insert(pos, nop)
                    pos += 1
                    i += 1
            i += 1


def build_nc():
    import concourse.bass as bass
    import concourse.mybir as mybir
    import concourse.tile as tile

    _patch_tile_drain()
    f32 = mybir.dt.float32
    bf16 = mybir.dt.bfloat16
    EXP = mybir.ActivationFunctionType.Exp

    nc = bass.Bass("TRN2", num_devices=NCORES)
    xT = nc.dram_tensor("xT", [C, N], bf16, kind="ExternalInput")
    Wq = nc.dram_tensor("Wq", [C, C], bf16, kind="ExternalInput")
    Wk = nc.dram_tensor("Wk", [C, C], bf16, kind="ExternalInput")
    Wv = nc.dram_tensor("Wv", [C, C], bf16, kind="ExternalInput")
    Wout = nc.dram_tensor("Wout", [C, C], bf16, kind="ExternalInput")
    bout = nc.dram_tensor("bout", [128, C], f32, kind="ExternalInput")
    y = nc.dram_tensor("y", [QH, C], f32, kind="ExternalOutput")

    with tile.TileContext(nc) as tc:
      with (
          tc.tile_pool(name="persist", bufs=1) as persist,
          tc.tile_pool(name="dramp", bufs=1, space="DRAM") as dram_pool,
      ):
        qT_sb = [persist.tile([128, QH], bf16, name=f"qT{j}", tag=f"qT{j}") for j in range(DT)]
        kT_sb = [persist.tile([128, N], bf16, name=f"kT{j}", tag=f"kT{j}") for j in range(DT)]
        v_sb = [persist.tile([128, H, DH + 1], bf16, name=f"v{m}", tag=f"v{m}") for m in range(MT)]
        rs_all = persist.tile([H, QH], f32)
        rinv_dram = dram_pool.tile([H, QH], f32)
        # aliases: after S of pair j consumes qT[j]/kT[j], their SBUF holds
        # the unnormalized O^T and the normalized A^T
        OT_un = qT_sb
        OT_fin = [kT_sb[j][:, 0:QH] for j in range(DT)]

        with (
            tc.tile_pool(name="projA", bufs=1) as projA,
            tc.tile_pool(name="ps_pp", bufs=2, space="PSUM") as pp,
        ):
            xT_t = projA.tile([128, CT, N], bf16)
            nc.sync.dma_start(
                out=xT_t, in_=xT.ap().rearrange("(a p) n -> p a n", p=128)
            )
            Wq_t = projA.tile([128, CT, C], bf16)
            nc.sync.dma_start(
                out=Wq_t, in_=Wq.ap().rearrange("(a p) d -> p a d", p=128)
            )
            Wk_t = projA.tile([128, CT, C], bf16)
            nc.sync.dma_start(
                out=Wk_t, in_=Wk.ap().rearrange("(a p) d -> p a d", p=128)
            )

            def emit_v(Wv_t, mt):
                nc.vector.memset(v_sb[mt][:, :, DH : DH + 1], 1.0)
                for ch in range(2):
                    psv = pp.tile([128, 512], f32, tag="ps", name=f"psv{mt}_{ch}")
                    for jc in range(CT):
                        nc.tensor.matmul(
                            psv,
                            xT_t[:, jc, mt * 128 : (mt + 1) * 128],
                            Wv_t[:, jc, ch * 512 : (ch + 1) * 512],
                            start=(jc == 0),
                            stop=(jc == CT - 1),
                        )
                    nc.vector.tensor_copy(
                        out=v_sb[mt][:, ch * 8 : (ch + 1) * 8, 0:DH],
                        in_=psv.rearrange("p (h d) -> p h d", h=8),
                    )

            def emit_q(jd, ch):
                psq = pp.tile([128, 512], f32, tag="ps", name=f"psq{jd}_{ch}")
                for jc in range(CT):
                    nc.tensor.matmul(
                        psq,
                        Wq_t[:, jc, jd * 128 : (jd + 1) * 128],
                        xT_t[:, jc, ch * 512 : (ch + 1) * 512],
                        start=(jc == 0),
                        stop=(jc == CT - 1),
                    )
                nc.vector.tensor_copy(
                    out=qT_sb[jd][:, ch * 512 : (ch + 1) * 512], in_=psq
                )

            def emit_k(jd, ch):
                psk = pp.tile([128, 512], f32, tag="ps", name=f"psk{jd}_{ch}")
                for jc in range(CT):
                    nc.tensor.matmul(
                        psk,
                        Wk_t[:, jc, jd * 128 : (jd + 1) * 128],
                        xT_t[:, jc, ch * 512 : (ch + 1) * 512],
                        start=(jc == 0),
                        stop=(jc == CT - 1),
                    )
                nc.vector.tensor_copy(
                    out=kT_sb[jd][:, ch * 512 : (ch + 1) * 512], in_=psk
                )

            def qk_ops(jd, half):
                # half 0: both q chunks + first k chunk; half 1: rest of k
                if half == 0:
                    return [lambda: emit_q(jd, 0), lambda: emit_q(jd, 1),
                            lambda: emit_k(jd, 0)]
                return [lambda ch=ch: emit_k(jd, ch) for ch in range(1, 4)]

            def emit_qk(jd, half):
                for op in qk_ops(jd, half):
                    op()

            with tc.tile_pool(name="projV", bufs=1) as projV:
                Wv_t = projV.tile([128, CT, C], bf16)
                nc.sync.dma_start(
                    out=Wv_t, in_=Wv.ap().rearrange("(a p) d -> p a d", p=128)
                )
                for mt in range(MT):
                    emit_v(Wv_t, mt)
            emit_qk(0, 0)
            emit_qk(0, 1)

            # ------------- attention (with woven q/k projections) ----------
            with (
                tc.tile_pool(name="aw_big", bufs=1) as awb,
                tc.tile_pool(name="aw_small", bufs=3) as aws,
                tc.tile_pool(name="ps_st", bufs=2, space="PSUM") as ps_st,
                tc.tile_pool(name="ps_ot", bufs=2, space="PSUM") as ps_ot,
            ):
                for pr in range(H // 2):      # head pairs; d-tile jd == pr
                    he, ho = 2 * pr, 2 * pr + 1
                    for qc in range(NQC):
                        qs = slice(qc * 512, (qc + 1) * 512)
                        pt = awb.tile([128, MT, 1024], bf16, tag="pt")
                        ot_e = ps_ot.tile([65, 512], f32, tag="ot")
                        ot_o = ps_ot.tile([65, 512], f32, tag="ot")
                        for mt in range(MT):
                            ms = slice(mt * 128, (mt + 1) * 128)
                            st = ps_st.tile([128, 1024], f32, tag="st")
                            # S^T tiles for both heads, row-packed (K=64)
                            nc.tensor.matmul(
                                st[:, 0:512],
                                kT_sb[pr][0:64, ms],
                                qT_sb[pr][0:64, qs],
                                start=True, stop=True,
                                tile_position=(0, 0),
                            )
                            nc.tensor.matmul(
                                st[:, 512:1024],
                                kT_sb[pr][64:128, ms],
                                qT_sb[pr][64:128, qs],
                                start=True, stop=True,
                                tile_position=(64, 0),
                            )
                            nc.scalar.activation(out=pt[:, mt, :], in_=st, func=EXP)
                            nc.tensor.matmul(
                                ot_e,
                                v_sb[mt][:, he, :],
                                pt[:, mt, 0:512],
                                start=(mt == 0), stop=(mt == MT - 1),
                            )
                            nc.tensor.matmul(
                                ot_o,
                                v_sb[mt][:, ho, :],
                                pt[:, mt, 512:1024],
                                start=(mt == 0), stop=(mt == MT - 1),
                            )
                        # stash unnormalized O^T (bf16, aliases qT) + rowsums
                        for po, h, ot in ((0, he, ot_e), (64, ho, ot_o)):
                            tmp = aws.tile([64, 512], bf16, tag="tmp")
                            nc.vector.tensor_copy(out=tmp, in_=ot[0:64, :])
                            nc.sync.dma_start(
                                out=OT_un[pr][po : po + 64, qs], in_=tmp
                            )
                            rsv = aws.tile([65, 512], f32, tag="rsv")
                            nc.vector.tensor_copy(
                                out=rsv[64:65, :], in_=ot[64:65, :]
                            )
                            nc.sync.dma_start(
                                out=rs_all[h : h + 1, qs], in_=rsv[64:65, :]
                            )
                        # weave the next pair's q/k projection
                        if pr + 1 < DT:
                            emit_qk(pr + 1, qc)
                        if pr == H // 2 - 1 and qc == 0:
                            # cols 0:512 rowsums are complete for all pairs:
                            # normalize that half now (DVE/DMA only), hidden
                            # under this last unit, so Y can start warm.
                            rinv0 = aws.tile([16, 512], f32, tag="rsv")
                            nc.vector.reciprocal(
                                out=rinv0, in_=rs_all[:, 0:512]
                            )
                            nc.sync.dma_start(
                                out=rinv_dram[:, 0:512], in_=rinv0
                            )
                            for jd in range(DT - 1):
                                rbc0 = aws.tile([128, 512], f32, tag="rsv")
                                nc.sync.dma_start(
                                    out=rbc0[0:64, :],
                                    in_=rinv_dram[2 * jd : 2 * jd + 1, 0:512].to_broadcast([64, 512]),
                                )
                                nc.sync.dma_start(
                                    out=rbc0[64:128, :],
                                    in_=rinv_dram[2 * jd + 1 : 2 * jd + 2, 0:512].to_broadcast([64, 512]),
                                )
                                nc.vector.tensor_mul(
                                    OT_fin[jd][:, 0:512],
                                    OT_un[jd][:, 0:512],
                                    rbc0,
                                )

        # ------------- remaining normalization + output projection ---------
        with (
            tc.tile_pool(name="normp", bufs=2) as np_pool,
            tc.tile_pool(name="yout", bufs=2) as yp,
            tc.tile_pool(name="ps_y", bufs=2, space="PSUM") as ps_y,
        ):
            Wout_t = yp.tile([128, DT, C], bf16, bufs=1)
            nc.sync.dma_start(
                out=Wout_t, in_=Wout.ap().rearrange("(a p) d -> p a d", p=128)
            )
            bout_t = yp.tile([128, C], f32, bufs=1)
            nc.sync.dma_start(out=bout_t, in_=bout.ap())

            def norm_half(jd, lo, hi):
                rbc = np_pool.tile([128, 512], f32, tag="rbc")
                nc.sync.dma_start(
                    out=rbc[0:64, :],
                    in_=rinv_dram[2 * jd : 2 * jd + 1, lo:hi].to_broadcast([64, 512]),
                )
                nc.sync.dma_start(
                    out=rbc[64:128, :],
                    in_=rinv_dram[2 * jd + 1 : 2 * jd + 2, lo:hi].to_broadcast([64, 512]),
                )
                nc.vector.tensor_mul(
                    OT_fin[jd][:, lo:hi], OT_un[jd][:, lo:hi], rbc
                )

            norm_half(DT - 1, 0, 512)     # pair 7 half0 (kT[7] now free)
            rinv1 = np_pool.tile([16, 512], f32, bufs=2, tag="rinv")
            nc.vector.reciprocal(out=rinv1, in_=rs_all[:, 512:1024])
            nc.sync.dma_start(out=rinv_dram[:, 512:1024], in_=rinv1)
            for jd in range(DT):
                norm_half(jd, 512, 1024)

            for t in range(QH // 128):
                psy = ps_y.tile([128, C], f32, tag="y")
                for jd in range(DT):
                    for ch in range(2):
                        nc.tensor.matmul(
                            psy[:, ch * 512 : (ch + 1) * 512],
                            OT_fin[jd][:, t * 128 : (t + 1) * 128],
                            Wout_t[:, jd, ch * 512 : (ch + 1) * 512],
                            start=(jd == 0),
                            stop=(jd == DT - 1),
                        )
                ys = yp.tile([128, C], f32, tag="ys")
                nc.vector.tensor_add(ys, psy, bout_t)
                nc.sync.dma_start(out=y[t * 128 : (t + 1) * 128, :], in_=ys)
    _split_excess_waits(nc)
    return nc


def make_in_maps(x, Wq, Wkv, Wout, bout):
    x = np.asarray(x, dtype=np.float32)
    Wq = np.asarray(Wq, dtype=np.float32)
    Wkv = np.asarray(Wkv, dtype=np.float32)
    Wout = np.asarray(Wout, dtype=np.float32)
    bout = np.asarray(bout, dtype=np.float32)
    Wq_b = np.ascontiguousarray((Wq * SCALE)).astype(_BF16)
    Wk_b = np.ascontiguousarray(Wkv[:, :C]).astype(_BF16)
    Wv_b = np.ascontiguousarray(Wkv[:, C:]).astype(_BF16)
    Wout_b = np.ascontiguousarray(Wout).astype(_BF16)
    bout_bc = np.ascontiguousarray(np.broadcast_to(bout, (128, C))).astype(np.float32)
    in_maps = []
    for core in range(NCORES):
        b, g = core // 2, core % 2
        # rotate so this core's query half is always columns 0:QH of xT
        xrot = np.concatenate(
            [x[b, g * QH : (g + 1) * QH], x[b, (1 - g) * QH : (2 - g) * QH]],
            axis=0,
        )
        xT_r = np.ascontiguousarray(xrot.T).astype(_BF16)
        in_maps.append(
            dict(xT=xT_r, Wq=Wq_b, Wk=Wk_b, Wv=Wv_b, Wout=Wout_b, bout=bout_bc)
        )
    return in_maps


def assemble(results):
    out = np.empty((B, N, C), dtype=np.float32)
    for core in range(NCORES):
        b, g = core // 2, core % 2
        out[b, g * QH : (g + 1) * QH, :] = results[core]["y"]
    return out


def kernel(x, Wq, Wkv, Wout, bout):
    from concourse.bass_utils import run_bass_kernel_spmd

    if "nc" not in _cache:
        _cache["nc"] = build_nc()
    in_maps = make_in_maps(x, Wq, Wkv, Wout, bout)
    res = run_bass_kernel_spmd(_cache["nc"], in_maps, core_ids=list(range(NCORES)))
    return assemble(res.results)
